# revision 21
# baseline (speedup 1.0000x reference)
"""MoE layer (B=8192, D=1024, E=8, top-2, H=2048) on 8 TRN2 NeuronCores.

Strategy (expert-parallel with two-segment load balancing):
  - Host: gate logits = x @ Wg (fp32), exact top-2 (jax tie-break semantics),
    softmax over the 2 picks. The 16384 (token, expert) pairs are grouped by
    expert; a small DP picks segment sizes (S1 >= S2) and cuts experts into
    at most 8 S1-pieces and 8 S2-pieces (one of each per core) minimizing
    C = S1 + S2, so every core carries ~the mean load.
  - Device (SPMD): each core runs two segments; segment rows use that
    segment's expert weights: y = relu(x @ W1[e] + b1[e]) @ W2[e] in bf16
    with fp32 PSUM accumulation, bf16 output. Weights resident in SBUF,
    tokens processed in column chunks of <=512.
  - Host: weighted combine out[tok] = sum_k gate * (y + b2[e]).

DMA lessons baked in:
  - One dma_start spreads across all 16 SDMA engines; the HWDGE rings (SP,
    ACT) are FIFO per ring and fair-share HBM against each other.
  - Issue cost ~ descriptor count (one per contiguous run, serialized on the
    issuing sequencer). All DRAM staging layouts below are partition-major so
    every piece is contiguous per partition -> 128 descriptors per dma_start.
  - The critical chunk-0 stream (w1a m-major, then w2a k2-major) owns the ACT
    ring; x/b1/outputs ride SP; bulk (late x, segment-B weights) issues after
    chunk-0 is fed.
"""

import os

import numpy as np
import ml_dtypes

B, D, E, TOP_K = 8192, 1024, 8, 2
H = 2 * D
P = 128
CHUNK = 512

KD = D // P  # 8 contraction tiles for mm1 (over D)
MH = H // P  # 16 output tiles for mm1 / contraction tiles for mm2 (over H)
MD = D // P  # 8 output tiles for mm2 (over D)

WARM_MMS = 54  # junk 128x128 MMs bridging framework preamble -> first data

_BF16 = np.dtype(ml_dtypes.bfloat16)

LAST_RESULTS = None  # BassKernelResults of the most recent run (for test harness)


def _chunk_sizes(n):
    """Split n columns into matmul chunks <=512, avoiding tiny tails."""
    chunks = [CHUNK] * (n // CHUNK)
    tail = n % CHUNK
    if tail >= 128 or not chunks:
        if tail:
            chunks.append(tail)
    elif tail:
        last = chunks.pop() + tail
        chunks += [last - last // 2, last // 2]
    return chunks


def _chunk_list(S1, S2):
    out = []
    off = 0
    for seg, seg_len in ((0, S1), (1, S2)):
        for tw in _chunk_sizes(seg_len):
            out.append((seg, off, tw))
            off += tw
    # End on the smallest chunk so the final copy+DMA trail is short. (Keep
    # chunk 0 first: the DMA critical path is tuned for it.)
    first, rest = out[0], out[1:]
    tail = min(rest, key=lambda c: c[2])
    rest.remove(tail)
    return [first] + rest + [tail]


def _feasible(counts, S1, S2, max_bins=3):
    """Can counts be covered by <=8 bins of S1 and <=8 of S2, each bin holding
    one contiguous piece of one expert? Returns per-expert (a, b) bin usage
    or None."""
    states = {(0, 0): []}  # (s1_used, s2_used) -> [(a, b) per expert]
    for c in counts:
        nxt = {}
        for (ua, ub), hist in states.items():
            for a in range(0, max_bins + 1):
                for b in range(0, max_bins + 1 - a):
                    if a + b == 0 or a * S1 + b * S2 < c:
                        continue
                    k = (ua + a, ub + b)
                    if k[0] > 8 or k[1] > 8 or k in nxt:
                        continue
                    nxt[k] = hist + [(a, b)]
        # prune dominated states
        keep = {}
        for k in sorted(nxt):
            if not any(o[0] <= k[0] and o[1] <= k[1] and o != k for o in nxt):
                keep[k] = nxt[k]
        states = keep or nxt
        if not states:
            return None
    return min(states.values(), key=lambda h: 0)


def _plan_segments(counts):
    """Minimize C = S1 + S2 (S1 >= S2) s.t. every expert's pair list can be
    cut into pieces filling 8 S1-bins and 8 S2-bins (one of each per core).
    Returns (S1, S2, pieces); pieces[core] = [(expert, tok_start, len) x2]."""
    counts = [int(c) for c in counts]
    # quick upper bound: every expert split in half across two same-type bins
    hi = 2 * max(-(-c // 2) for c in counts)
    best = None
    for S1 in range(1024, hi + 2, 2):
        if best is not None and S1 >= best[0]:
            break
        lo, hh = max(0, 2048 - S1), min(S1, (best[0] - S1 - 2) if best else S1)
        if lo > hh:
            continue
        # binary search smallest even S2 in [lo, hh]
        lo, hh = -(-lo // 2), hh // 2
        found = None
        while lo <= hh:
            mid = (lo + hh) // 2
            ab = _feasible(counts, S1, 2 * mid)
            if ab is not None:
                found = (2 * mid, ab)
                hh = mid - 1
            else:
                lo = mid + 1
        if found and (best is None or S1 + found[0] < best[0]):
            best = (S1 + found[0], S1, found[0], found[1])
    assert best is not None
    _, S1, S2, ab = best

    s1_pieces, s2_pieces = [], []
    for e, (c, (a, b)) in enumerate(zip(counts, ab)):
        off = 0
        for i in range(a):
            ln = min(S1, c - off)
            min_ln = max(0, (c - off) - ((a - 1 - i) * S1 + b * S2))
            ln = max(ln, min_ln)
            s1_pieces.append((e, off, ln))
            off += ln
        rem = c - off
        for i in range(b):
            ln = min(S2, -(-rem // (b - i)))
            s2_pieces.append((e, off, ln))
            off += ln
            rem -= ln
        assert rem == 0 and off == c, (e, c, a, b, off)
    while len(s1_pieces) < E:
        s1_pieces.append((0, 0, 0))
    while len(s2_pieces) < E:
        s2_pieces.append((0, 0, 0))
    assert all(ln <= S1 for _, _, ln in s1_pieces)
    assert all(ln <= S2 for _, _, ln in s2_pieces)
    pieces = [[s1_pieces[i], s2_pieces[i]] for i in range(E)]
    return S1, S2, pieces


def _build_program(S1, S2):
    import concourse.bacc as bacc
    import concourse.mybir as mybir
    import concourse.tile as tile
    from concourse.bass import ts

    C = S1 + S2
    nc = bacc.Bacc("TRN2", target_bir_lowering=False, debug=False)
    bf16 = mybir.dt.bfloat16
    f32 = mybir.dt.float32

    chunk_list = _chunk_list(S1, S2)

    # All input staging layouts are partition-major (contiguous per partition
    # on both sides of every planned dma piece).
    xc_ds = [
        nc.dram_tensor(f"xt{i}", (P, KD, tw), bf16, kind="ExternalInput").ap()
        for i, (_, _, tw) in enumerate(chunk_list)
    ]
    w1_ds, w2_ds = [], []
    for s in ("a", "b"):
        w1_ds.append(
            nc.dram_tensor(f"w1{s}", (P, MH, KD, P), bf16, kind="ExternalInput").ap()
        )
        w2_ds.append(
            nc.dram_tensor(f"w2{s}", (P, MH, MD, P), bf16, kind="ExternalInput").ap()
        )
    b1_d = nc.dram_tensor("b1r", (P, 2, MH), f32, kind="ExternalInput").ap()
    yt_d = nc.dram_tensor("yt", (D, C), bf16, kind="ExternalOutput").ap()

    with tile.TileContext(nc) as tc:
        with (
            tc.tile_pool(name="weights", bufs=1) as wpool,
            tc.tile_pool(name="xin", bufs=1) as xpool,
            tc.tile_pool(name="hbuf", bufs=1) as hpool,
            tc.tile_pool(name="ystage", bufs=3) as ypool,
            tc.tile_pool(name="ps", bufs=8, space="PSUM") as pspool,
        ):
            xc_sbs = [
                xpool.tile([P, KD, tw], bf16, name=f"xc{i}")
                for i, (_, _, tw) in enumerate(chunk_list)
            ]
            w1_sbs = [
                wpool.tile([P, MH, KD, P], bf16, name=f"w1sb{s}") for s in range(2)
            ]
            w2_sbs = [
                wpool.tile([P, MH, MD, P], bf16, name=f"w2sb{s}") for s in range(2)
            ]
            b1_sb = wpool.tile([P, 2, MH], f32, name="b1sb")

            # --- DMA schedule (see module docstring) ---
            # The chunk-0 critical stream is interleaved across BOTH rings by
            # consumption deadline: mm1-c0 eats w1a at ~145GB/s while one
            # ring's fair share is only ~180GB/s, so a single-ring stream has
            # zero slack. SP carries x + the w1a/w2a pieces whose deadlines
            # tolerate queueing behind x; ACT carries the earliest pieces.
            nc.scalar.dma_start(w1_sbs[0][:, 0:2], w1_ds[0][:, 0:2])
            nc.scalar.dma_start(w1_sbs[0][:, 2:4], w1_ds[0][:, 2:4])
            nc.scalar.dma_start(w1_sbs[0][:, 6:8], w1_ds[0][:, 6:8])
            nc.scalar.dma_start(w1_sbs[0][:, 12:16], w1_ds[0][:, 12:16])
            nc.scalar.dma_start(w2_sbs[0][:, 0:4], w2_ds[0][:, 0:4])
            nc.scalar.dma_start(w2_sbs[0][:, 4:8], w2_ds[0][:, 4:8])
            # SP ring: chunk-0 x first (k-halves: chunk-0 mm1 starts on the
            # first half), bias, then mid-deadline weight pieces.
            nc.sync.dma_start(xc_sbs[0][:, 0:4], xc_ds[0][:, 0:4])
            nc.sync.dma_start(xc_sbs[0][:, 4:8], xc_ds[0][:, 4:8])
            nc.sync.dma_start(b1_sb, b1_d)
            nc.sync.dma_start(w1_sbs[0][:, 4:6], w1_ds[0][:, 4:6])
            nc.sync.dma_start(w1_sbs[0][:, 8:12], w1_ds[0][:, 8:12])
            nc.sync.dma_start(w2_sbs[0][:, 8:16], w2_ds[0][:, 8:16])

            def mm2_phase(seg, off, tw, h_sb, k2_outer):
                # k2-outer: 4-bank halves accumulate together; half-0's
                # copies overlap half-1's matmuls. The last chunk uses
                # m2-outer instead so its copies/output DMAs overlap its own
                # matmul stream rather than trailing it.
                w2_sb = w2_sbs[seg]
                if k2_outer:
                    for m2_base in (0, MD // 2):
                        m2s = range(m2_base, m2_base + MD // 2)
                        pys = {
                            m2: pspool.tile(
                                [P, CHUNK], f32, tag="ps", name=f"py{m2}"
                            )
                            for m2 in m2s
                        }
                        for k2 in range(MH):
                            for m2 in m2s:
                                nc.tensor.matmul(
                                    pys[m2][:, :tw],
                                    w2_sb[:, k2, m2],
                                    h_sb[:, k2, :tw],
                                    start=(k2 == 0),
                                    stop=(k2 == MH - 1),
                                )
                        for m2 in m2s:
                            y_sb = ypool.tile([P, CHUNK], bf16, tag="y")
                            nc.vector.tensor_copy(y_sb[:, :tw], pys[m2][:, :tw])
                            nc.sync.dma_start(
                                yt_d[ts(m2, P), off : off + tw], y_sb[:, :tw]
                            )
                else:
                    # Final chunk: m2-outer; spread output DMAs across both
                    # rings (ACT is idle by now) and split the very last tile
                    # so the trailing transfer after the last matmul is short.
                    for m2 in range(MD):
                        py = pspool.tile([P, CHUNK], f32, tag="ps", name="py")
                        for k2 in range(MH):
                            nc.tensor.matmul(
                                py[:, :tw],
                                w2_sb[:, k2, m2],
                                h_sb[:, k2, :tw],
                                start=(k2 == 0),
                                stop=(k2 == MH - 1),
                            )
                        y_sb = ypool.tile([P, CHUNK], bf16, tag="y")
                        if m2 == MD - 1:
                            # Split the final tile's cast+store so the first
                            # half's DMA issues while the second half casts.
                            half = (tw // 2 + 1) & ~1
                            nc.vector.tensor_copy(y_sb[:, :half], py[:, :half])
                            nc.sync.dma_start(
                                yt_d[ts(m2, P), off : off + half],
                                y_sb[:, :half],
                            )
                            nc.vector.tensor_copy(
                                y_sb[:, half:tw], py[:, half:tw]
                            )
                            nc.scalar.dma_start(
                                yt_d[ts(m2, P), off + half : off + tw],
                                y_sb[:, half:tw],
                            )
                        else:
                            nc.vector.tensor_copy(y_sb[:, :tw], py[:, :tw])
                            ring = nc.scalar if m2 % 2 else nc.sync
                            ring.dma_start(
                                yt_d[ts(m2, P), off : off + tw], y_sb[:, :tw]
                            )

            # PE warmup: junk matmuls on a memset tile run while the first
            # weight/activation DMAs land, so the HAM clock gate ramps toward
            # 8/8 before real matmuls start.
            # warm_ps comes from the main PSUM pool: after the warmup matmuls
            # its bank recycles into the 8-deep rotation (no fills may touch
            # it later — chunk transitions need all 8 banks to keep the next
            # chunk's mm1 from waiting on cast drains).
            warm_sb = xpool.tile([P, P], bf16, name="warm")
            nc.vector.memset(warm_sb, 0.0)
            warm_ps = pspool.tile([P, P], f32, tag="ps", name="warm_ps")
            for _ in range(WARM_MMS):
                nc.tensor.matmul(warm_ps, warm_sb, warm_sb, start=True, stop=True)

            for ci, (seg, off, tw) in enumerate(chunk_list):
                if ci == 1:
                    # Bulk transfers: remaining x chunks (SP) and segment-B
                    # weights (ACT), issued once chunk-0's stream has drained.
                    for j in range(1, len(chunk_list)):
                        nc.sync.dma_start(xc_sbs[j], xc_ds[j])
                    nc.scalar.dma_start(w1_sbs[1], w1_ds[1])
                    nc.scalar.dma_start(w2_sbs[1], w2_ds[1])
                w1_sb = w1_sbs[seg]
                x_sb = xc_sbs[ci]
                h_sb = hpool.tile([P, MH, CHUNK], bf16, tag="h")
                if ci == 0:
                    # Chunk 0 runs mm1 in k-halves over 4-m groups: the first
                    # 16 matmuls need only x[k<4] + w1a[m<4], so real compute
                    # starts after ~0.75MB of DMA instead of ~3MB.
                    for mb in range(0, MH, 4):
                        phs = {
                            m: pspool.tile([P, CHUNK], f32, tag="ps", name="ph")
                            for m in range(mb, mb + 4)
                        }
                        for kh in (0, KD // 2):
                            for m in phs:
                                for k in range(kh, kh + KD // 2):
                                    nc.tensor.matmul(
                                        phs[m][:, :tw],
                                        w1_sb[:, m, k],
                                        x_sb[:, k, :],
                                        start=(k == 0),
                                        stop=(k == KD - 1),
                                    )
                        for m in phs:
                            nc.scalar.activation(
                                h_sb[:, m, :tw],
                                phs[m][:, :tw],
                                mybir.ActivationFunctionType.Relu,
                                bias=b1_sb[:, seg, m : m + 1],
                            )
                else:
                    for m in range(MH):
                        ph = pspool.tile([P, CHUNK], f32, tag="ps", name="ph")
                        for k in range(KD):
                            nc.tensor.matmul(
                                ph[:, :tw],
                                w1_sb[:, m, k],
                                x_sb[:, k, :],
                                start=(k == 0),
                                stop=(k == KD - 1),
                            )
                        nc.scalar.activation(
                            h_sb[:, m, :tw],
                            ph[:, :tw],
                            mybir.ActivationFunctionType.Relu,
                            bias=b1_sb[:, seg, m : m + 1],
                        )
                mm2_phase(seg, off, tw, h_sb, k2_outer=(ci < len(chunk_list) - 1))
    nc.finalize()
    return nc


def _route(x, Wg):
    """Exact reference gating on host: top-2 of clean fp32 logits (jax
    tie-break: lower index first), softmax over the two picks."""
    logits = x @ Wg  # [B, E] fp32
    order = np.argsort(-logits, axis=1, kind="stable")[:, :TOP_K]  # [B, 2]
    top_vals = np.take_along_axis(logits, order, axis=1)
    ex = np.exp(top_vals - top_vals[:, :1])  # top_vals sorted desc -> max first
    gates = (ex / ex.sum(axis=1, keepdims=True)).astype(np.float32)  # [B, 2]
    return order, gates


def kernel(x, Wg, W1, b1, W2, b2):
    x = np.ascontiguousarray(np.asarray(x, dtype=np.float32))
    Wg = np.asarray(Wg, dtype=np.float32)
    W1 = np.asarray(W1, dtype=np.float32)
    b1 = np.asarray(b1, dtype=np.float32)
    W2 = np.asarray(W2, dtype=np.float32)
    b2 = np.asarray(b2, dtype=np.float32)

    order, gates = _route(x, Wg)

    # Dispatch: flatten (token, k) pairs, bucket by expert (stable => slot
    # order within an expert follows token order). Pair p belongs to token p//2.
    expert_flat = order.reshape(-1)  # [2B]
    gate_flat = gates.reshape(-1)  # [2B]
    perm = np.argsort(expert_flat, kind="stable")  # pairs grouped by expert
    counts = np.bincount(expert_flat, minlength=E)
    offs = np.concatenate(([0], np.cumsum(counts)))[:E]

    S1, S2, pieces = _plan_segments(counts)
    C = S1 + S2
    assert C <= 4864, f"unexpectedly imbalanced routing: {counts}"
    chunk_list = _chunk_list(S1, S2)

    # Per-pair placement (core, column) for the combine step, and per-core
    # token lists for the dispatch.
    core_of_pair = np.empty(2 * B, dtype=np.int64)
    col_of_pair = np.empty(2 * B, dtype=np.int64)
    xT = np.ascontiguousarray(x.T)  # [D, B]
    w1_pm = [None] * E  # partition-major bf16 weight caches
    w2_pm = [None] * E
    in_maps = []
    for core in range(E):
        xg = np.zeros((D, C), dtype=_BF16)
        in_map = {}
        b1r = np.zeros((P, 2, MH), dtype=np.float32)
        for seg, (e, tok_start, ln) in enumerate(pieces[core]):
            seg_off = 0 if seg == 0 else S1
            if ln:
                pair_idx = perm[offs[e] + tok_start : offs[e] + tok_start + ln]
                toks = pair_idx // 2
                xg[:, seg_off : seg_off + ln] = xT[:, toks].astype(_BF16)
                core_of_pair[pair_idx] = core
                col_of_pair[pair_idx] = seg_off + np.arange(ln)
            if w1_pm[e] is None:
                # [d, h] -> [p, m, ko, c] with d = ko*P + p, h = m*P + c
                w1_pm[e] = np.ascontiguousarray(
                    W1[e].astype(_BF16).reshape(KD, P, MH, P).transpose(1, 2, 0, 3)
                )
                # [h, d] -> [p, k2, m2, c] with h = k2*P + p, d = m2*P + c
                w2_pm[e] = np.ascontiguousarray(
                    W2[e].astype(_BF16).reshape(MH, P, MD, P).transpose(1, 0, 2, 3)
                )
            s = "ab"[seg]
            in_map[f"w1{s}"] = w1_pm[e]
            in_map[f"w2{s}"] = w2_pm[e]
            b1r[:, seg, :] = b1[e].reshape(MH, P).T
        in_map["b1r"] = b1r
        # x chunks, partition-major: [d, c] -> [p, ko, c]
        xp = xg.reshape(KD, P, C).transpose(1, 0, 2)
        for i, (_, off, tw) in enumerate(chunk_list):
            in_map[f"xt{i}"] = np.ascontiguousarray(xp[:, :, off : off + tw])
        in_maps.append(in_map)

    nc = _build_program(S1, S2)

    from concourse.bass_utils import run_bass_kernel_spmd

    trace = os.environ.get("MOE_TRACE") == "1"
    kwargs = {}
    if trace:
        kwargs = dict(trace=True, trace_cores=list(range(E)))
    try:
        res = run_bass_kernel_spmd(nc, in_maps, core_ids=list(range(E)), **kwargs)
    except Exception:  # wedged accelerator: reset once and retry untraced
        try:
            import ctypes

            lib = ctypes.CDLL("/opt/axon/libaxon_pjrt.so")
            lib.axon_reset.restype = ctypes.c_int64
            lib.axon_reset()
        except OSError:
            pass
        res = run_bass_kernel_spmd(nc, in_maps, core_ids=list(range(E)))
    global LAST_RESULTS
    LAST_RESULTS = res

    Y = np.stack([np.asarray(r["yt"], dtype=np.float32) for r in res.results])

    # Combine: pair p contributes gate_p * (y[:, col_p] + b2[e_p]) to token
    # p//2. Pairs of token b sit at flat positions 2b, 2b+1.
    cols = Y[core_of_pair, :, col_of_pair]  # [2B, D]
    weighted = (cols + b2[expert_flat]) * gate_flat[:, None]
    out = weighted[0::2] + weighted[1::2]
    return np.ascontiguousarray(out, dtype=np.float32)
